# revision 11
# baseline (speedup 1.0000x reference)
"""GPTQ 4-bit quant linear (nn_Autograd4bitQuantLinear) on 8 TRN2 NeuronCores.

Strategy (column-parallel tensor parallelism, per sharding hint):
 - Host: dequantize packed 4-bit weights to W [4096, 11008] f32, shard along
   out_features (1376 per core). x transposed to xT [4096, 8192] (contraction
   on partitions), replicated across cores.
 - Device (per core): xT.T @ W_shard on the PE, fp32 PSUM accumulation.
   W shard stays resident in SBUF; x streams in 512-token blocks; psum chunks
   of 512/512/352 out-features; out [8192, 1376] written back (fp16 device
   side, upcast to f32 on host).
 - Host: concatenate the 8 shards along the last dim.

Precision plan: the 32 k-tiles of the contraction are split into three
segments, scaled so every segment's (x-scale * W-scale) product is S=16384
(descaled once at PSUM eviction):
 - NX3 head tiles: x in fp8-e3m4 (xT * 2, exact-range +-15.5), W in fp16
   (* 8192). Halves x HBM traffic at ~1/4 the quantization error variance
   of e4m3 (4 mantissa bits), PE runs at normal fp16 rate.
 - middle tiles: x fp16 (*32), W fp16 (*512) - full precision.
 - 2*N8 tail tiles: x and W in fp8-e4m3 (*32 / *512, clipped to +-240, the
   TRN e4m3 range), run as N8 DoubleRow pairs (2 MACs/cell/cycle).
Simulated rel err (bit-exact host model): NX3=32/N8=0 -> 1.37e-2,
NX3=28/N8=2 -> 1.85e-2, NX3=0/N8=4 -> 1.87e-2; gate is 2e-2.
Measured on the 8-core terminal (interleaved repeat-differencing): the
kernel is PE-compute-bound; time ranks by PE columns streamed, so NX3=0 /
N8=4 (max DoubleRow under the error gate) is fastest: ~629us/rep vs the
~957us/rep fp32-out fp16 baseline, device rel err 1.8716e-2.
"""

import os
import numpy as np
import ml_dtypes

IN_F = 4096
OUT_F = 11008
GROUP = 128
TOKENS = 8192
NCORES = 8
SHARD = OUT_F // NCORES  # 1376
P = 128
KT = IN_F // P  # 32 k-tiles
TB = 512  # tokens per block
NBLK = TOKENS // TB  # 16
TSUB = TB // P  # 4
CHUNKS = [(0, 512), (512, 512), (1024, SHARD - 1024)]  # psum-bank sized chunks

# ---- precision configuration ----
MM_DT = "float16"  # dtype of the full-precision k-tiles: float16 or bfloat16
NX3 = 0  # head k-tiles with x in fp8-e3m4 (W stays fp16)
N8 = 4  # fp8-e4m3 DoubleRow k-tile PAIRS at the tail (each covers 2 k-tiles)
OUT_DT = "float16"  # device-side out dtype: float32 or float16 (host upcasts)
SX3 = 2.0  # e3m4 x pre-scale
SXB = 32.0  # fp16 x pre-scale
SX8 = 32.0  # e4m3 x pre-scale
S_ALL = 16384.0  # uniform (x*W) scale product, descaled at eviction

_CACHE = {}


def _cfg():
    return (MM_DT, NX3, N8, OUT_DT)


def _seg():
    nb = KT - NX3 - 2 * N8  # fp16-x k-tiles
    assert nb >= 0
    scaled = not (NX3 == 0 and N8 == 0)
    return nb, (S_ALL if scaled else 1.0)


def _build_nc(reps=1):
    import concourse.bass as bass
    import concourse.mybir as mybir
    import concourse.tile as tile
    from concourse import bacc

    nb, scale = _seg()
    nw16 = NX3 + nb  # k-tiles whose W lives in fp16

    nc = bacc.Bacc(
        "TRN2",
        target_bir_lowering=False,
        debug=False,
        enable_asserts=False,
        num_devices=NCORES,
    )
    mdt = getattr(mybir.dt, MM_DT)
    f83 = mybir.dt.float8e3
    f8 = mybir.dt.float8e4
    f32 = mybir.dt.float32
    odt = getattr(mybir.dt, OUT_DT)
    DR = mybir.MatmulPerfMode.DoubleRow

    xt3 = xt16 = w16 = xt8 = w8 = None
    if NX3 > 0:
        xt3 = nc.dram_tensor("xt3", [NX3 * P, TOKENS], f83, kind="ExternalInput").ap()
    if nb > 0:
        xt16 = nc.dram_tensor("xt16", [nb * P, TOKENS], mdt, kind="ExternalInput").ap()
    if nw16 > 0:
        w16 = nc.dram_tensor("w16", [nw16 * P, SHARD], mdt, kind="ExternalInput").ap()
    if N8 > 0:
        xt8 = nc.dram_tensor("xt8", [2 * N8 * P, TOKENS], f8, kind="ExternalInput").ap()
        w8 = nc.dram_tensor("w8", [2 * N8 * P, SHARD], f8, kind="ExternalInput").ap()
    out = nc.dram_tensor("out", [TOKENS, SHARD], odt, kind="ExternalOutput").ap()

    with tile.TileContext(nc) as tc:
        with (
            tc.tile_pool(name="wp", bufs=1) as wp,
            tc.tile_pool(name="xp", bufs=3) as xp,
            tc.tile_pool(name="op", bufs=3) as op,
            tc.tile_pool(name="pp", bufs=2, space=bass.MemorySpace.PSUM) as pp,
        ):
            if nw16 > 0:
                w16_sb = wp.tile([P, nw16, SHARD], mdt)
                for k in range(nw16):
                    nc.sync.dma_start(w16_sb[:, k, :], w16[k * P : (k + 1) * P, :])
            if N8 > 0:
                w8_sb = wp.tile([P, 2 * N8, SHARD], f8)
                for k in range(2 * N8):
                    nc.sync.dma_start(w8_sb[:, k, :], w8[k * P : (k + 1) * P, :])
            for _rep in range(reps):
                for b in range(NBLK):
                    if NX3 > 0:
                        x3_sb = xp.tile([P, NX3, TB], f83, name="x3")
                        for k in range(NX3):
                            nc.sync.dma_start(
                                x3_sb[:, k, :],
                                xt3[k * P : (k + 1) * P, b * TB : (b + 1) * TB],
                            )
                    if nb > 0:
                        x16_sb = xp.tile([P, nb, TB], mdt, name="x16")
                        for k in range(nb):
                            nc.sync.dma_start(
                                x16_sb[:, k, :],
                                xt16[k * P : (k + 1) * P, b * TB : (b + 1) * TB],
                            )
                    if N8 > 0:
                        x8_sb = xp.tile([P, 2 * N8, TB], f8, name="x8")
                        for k in range(2 * N8):
                            nc.sync.dma_start(
                                x8_sb[:, k, :],
                                xt8[k * P : (k + 1) * P, b * TB : (b + 1) * TB],
                            )
                    for s in range(TSUB):
                        o_sb = op.tile([P, SHARD], odt, name="o_sb")
                        pss = [
                            pp.tile([P, 512], f32, tag=f"ps{ci}", name=f"ps{ci}")
                            for ci in range(len(CHUNKS))
                        ]
                        nmm = NX3 + nb + N8  # matmuls per chunk
                        m = 0
                        for k in range(NX3):
                            lhsT = x3_sb[:, k, s * P : (s + 1) * P]
                            for ci, (n0, nw) in enumerate(CHUNKS):
                                nc.tensor.matmul(
                                    pss[ci][:, :nw],
                                    lhsT,
                                    w16_sb[:, k, n0 : n0 + nw],
                                    start=(m == 0),
                                    stop=(m == nmm - 1),
                                )
                            m += 1
                        for kb in range(nb):
                            lhsT = x16_sb[:, kb, s * P : (s + 1) * P]
                            for ci, (n0, nw) in enumerate(CHUNKS):
                                nc.tensor.matmul(
                                    pss[ci][:, :nw],
                                    lhsT,
                                    w16_sb[:, NX3 + kb, n0 : n0 + nw],
                                    start=(m == 0),
                                    stop=(m == nmm - 1),
                                )
                            m += 1
                        for j in range(N8):
                            lhsT = x8_sb[:, 2 * j : 2 * j + 2, s * P : (s + 1) * P]
                            for ci, (n0, nw) in enumerate(CHUNKS):
                                nc.tensor.matmul(
                                    pss[ci][:, :nw],
                                    lhsT,
                                    w8_sb[:, 2 * j : 2 * j + 2, n0 : n0 + nw],
                                    start=(m == 0),
                                    stop=(m == nmm - 1),
                                    perf_mode=DR,
                                )
                            m += 1
                        for ci, (n0, nw) in enumerate(CHUNKS):
                            if scale != 1.0:
                                nc.vector.tensor_scalar_mul(
                                    o_sb[:, n0 : n0 + nw], pss[ci][:, :nw], 1.0 / scale
                                )
                            else:
                                nc.vector.tensor_copy(
                                    o_sb[:, n0 : n0 + nw], pss[ci][:, :nw]
                                )
                        r0 = b * TB + s * P
                        nc.sync.dma_start(out[r0 : r0 + P, :], o_sb[:])
    nc.compile()
    return nc


def _dequant_f32(qweight, scales, qzeros, g_idx):
    """GPTQ v2 dequant: W = s * (w4 - (z4 + 1)), [in_features, out_features] f32."""
    shifts = np.arange(8, dtype=np.uint32) * 4
    qw = np.ascontiguousarray(qweight).view(np.uint32)
    w4 = (
        ((qw[:, None, :] >> shifts[None, :, None]) & np.uint32(0xF))
        .reshape(-1, qweight.shape[1])
        .astype(np.float32)
    )
    qz = np.ascontiguousarray(qzeros).view(np.uint32)
    z4 = (
        ((qz[:, :, None] >> shifts[None, None, :]) & np.uint32(0xF)).reshape(
            qzeros.shape[0], -1
        )
        + np.uint32(1)
    ).astype(np.float32)
    return scales[g_idx] * (w4 - z4[g_idx])


def prepare_in_maps(inputs):
    """Host-side input prep: dequant, scale, cast, shard. Returns per-core in_maps."""
    x = np.asarray(inputs["x"], dtype=np.float32)
    W = _dequant_f32(
        np.asarray(inputs["qweight"], dtype=np.int32),
        np.asarray(inputs["scales"], dtype=np.float32),
        np.asarray(inputs["qzeros"], dtype=np.int32),
        np.asarray(inputs["g_idx"], dtype=np.int32),
    )
    mdt = np.float16 if MM_DT == "float16" else ml_dtypes.bfloat16
    f83 = ml_dtypes.float8_e3m4
    f8 = ml_dtypes.float8_e4m3fn
    nb, scale = _seg()
    nw16 = NX3 + nb
    scaled = scale != 1.0
    r3 = NX3 * P  # e3m4-x rows
    rb = (NX3 + nb) * P  # end of fp16-x rows

    xt = np.ascontiguousarray(x.reshape(-1, IN_F).T)  # [IN_F, TOKENS] f32
    in_maps = [dict() for _ in range(NCORES)]
    if NX3 > 0:
        xt3 = np.ascontiguousarray(
            np.clip(xt[:r3] * SX3, -15.5, 15.5).astype(f83)
        )
        for m in in_maps:
            m["xt3"] = xt3
    if nb > 0:
        sx = SXB if scaled else 1.0
        xt16 = np.ascontiguousarray((xt[r3:rb] * sx).astype(mdt))
        for m in in_maps:
            m["xt16"] = xt16
    if N8 > 0:
        xt8 = np.ascontiguousarray(np.clip(xt[rb:] * SX8, -240, 240).astype(f8))
        for m in in_maps:
            m["xt8"] = xt8
    for c in range(NCORES):
        Wc = W[:, c * SHARD : (c + 1) * SHARD]
        if nw16 > 0:
            # per-segment W scale so every segment's x*W product scale is S_ALL
            w16 = np.empty((rb, SHARD), dtype=mdt)
            if NX3 > 0:
                w16[:r3] = (Wc[:r3] * (scale / SX3)).astype(mdt)
            if nb > 0:
                sw = (scale / SXB) if scaled else 1.0
                w16[r3:rb] = (Wc[r3:rb] * sw).astype(mdt)
            in_maps[c]["w16"] = w16
        if N8 > 0:
            in_maps[c]["w8"] = np.ascontiguousarray(
                np.clip(Wc[rb:] * (scale / SX8), -240, 240).astype(f8)
            )
    return in_maps


def kernel(x, qweight, scales, qzeros, g_idx):
    # NTFF tracing is unavailable under this axon client (antenv.axon_hooks
    # missing); force it off so a stray BASS_TRACE doesn't crash the run.
    os.environ["BASS_NEVER_TRACE"] = "1"
    from concourse.bass_utils import run_bass_kernel_spmd

    x = np.asarray(x, dtype=np.float32)
    in_maps = prepare_in_maps(
        {"x": x, "qweight": qweight, "scales": scales, "qzeros": qzeros, "g_idx": g_idx}
    )

    key = _cfg()
    if _CACHE.get("cfg") != key:
        _CACHE["nc"] = _build_nc()
        _CACHE["cfg"] = key
    nc = _CACHE["nc"]

    res = run_bass_kernel_spmd(nc, in_maps, core_ids=list(range(NCORES)), trace=False)
    _CACHE["last_results"] = res

    out = np.concatenate(
        [res.results[c]["out"].astype(np.float32) for c in range(NCORES)], axis=1
    )
    return np.ascontiguousarray(out.reshape(x.shape[0], x.shape[1], OUT_F))
